# revision 5
# baseline (speedup 1.0000x reference)
"""Bass/Trainium2 kernel for nn_GATModel (hetero 2-layer GAT, 8 relations).

Sharding: relation r -> NeuronCore r (8 relations, 8 cores).  The whole
layer runs on device: per-core projections (hs_ext = x_si @ [Ws|Wsv] in
bf16, ed = x_di @ Wdv), then an edge phase driven by host-prepacked
dst-sorted edge streams: dma_gather of hs_ext[src] / ed[dst] rows,
p = exp(leakyrelu(es+ed)) on ACT, per-128-edge-group segment sums via
one-hot matmul on PE (host-aligned so every destination row is written
by exactly one descriptor -- dma_scatter_add collides unsafely
otherwise), and dma_scatter_add into a chunked aggregate table.  A
node-level pass divides by the attention denominator and adds bias.
The inter-relation sum + ELU at layer boundaries runs on host between
the two launches of the same compiled program (segment-max of the
reference softmax is skipped: logits here are O(1) so the max-shift is
a mathematical no-op).

Self-contained: shapes/relations hardcoded; no sibling imports.
"""
import numpy as np

N = 100000
IN = 128
H = 4
C = 32
D = H * C
R = 8
E = 300000
REL = [(0, 1), (1, 0), (0, 2), (2, 0), (0, 3), (3, 0), (0, 4), (4, 0)]

CHUNK = 32640          # real node rows per int16-indexed table chunk (255*128)
TROW = 32767           # trash row inside each 32768-row chunk
NCHUNK = 4             # ceil(100000/32640) = 4
NI = 896               # tokens per SWDGE instruction (64-desc/lane limit)
SUB = NI // 128        # 7 subtiles per instruction
HSC = 192              # hs_ext table row elems (fp32): 128 hs + 4 es + pad
EDC = 64               # ed table row elems (fp32): 4 ed + pad
AGC = 192              # aggregate table row stride (fp32): 128 msg + 4 denom
NPAD = 782 * 128       # 100096, node count padded to 128

_CACHE = {}


def _pack_relation(src, dst):
    """Pack one relation's edges into per-instruction metadata.

    Returns list of (cd, cs, gidx[896]i16, edidx[896]i16, gval[896]f32,
    sidx[896]i16) in a fixed (cd, cs) bin order, plus per-bin instruction
    counts.  Within each bin edges are dst-sorted and whole dst-groups
    are packed into 128-token subtiles so each dst row is summed by one
    PE matmul and scattered exactly once.
    """
    order = np.argsort(dst, kind="stable")
    src, dst = src[order], dst[order]
    cd_all = dst // CHUNK
    cs_all = src // CHUNK
    out = []
    counts = {}
    for cd in range(NCHUNK):
        m_cd = cd_all == cd
        for cs in range(NCHUNK):
            m = m_cd & (cs_all == cs)
            s_loc = (src[m] - cs * CHUNK).astype(np.int16)
            d_loc = (dst[m] - cd * CHUNK).astype(np.int16)
            # dst-groups (d_loc sorted within bin)
            subtiles = []  # each: (g[128]i16 gidx, e[128]i16, gv[128], si[128])
            gi = np.zeros(128, np.int16)
            ei = np.zeros(128, np.int16)
            gv = np.arange(128, dtype=np.float32)
            si = np.full(128, TROW, np.int16)
            fill = 0
            ngr = 0
            bnd = np.nonzero(np.diff(d_loc))[0] + 1
            starts = np.concatenate(([0], bnd))
            ends = np.concatenate((bnd, [len(d_loc)]))
            for a, b in zip(starts, ends):
                gsz = b - a
                assert gsz <= 128, "dst group too large for a subtile"
                if fill + gsz > 128:
                    subtiles.append((gi, ei, gv, si))
                    gi = np.zeros(128, np.int16)
                    ei = np.zeros(128, np.int16)
                    gv = np.arange(128, dtype=np.float32)
                    si = np.full(128, TROW, np.int16)
                    fill = 0
                    ngr = 0
                gi[fill:fill + gsz] = s_loc[a:b]
                ei[fill:fill + gsz] = d_loc[a:b]
                gv[fill:fill + gsz] = ngr
                si[ngr] = d_loc[a]
                fill += gsz
                ngr += 1
            if fill or not subtiles:
                subtiles.append((gi, ei, gv, si))
            # pad subtile list to a multiple of SUB
            while len(subtiles) % SUB:
                subtiles.append((np.zeros(128, np.int16), np.zeros(128, np.int16),
                                 np.arange(128, dtype=np.float32),
                                 np.full(128, TROW, np.int16)))
            n_ins = len(subtiles) // SUB
            counts[(cd, cs)] = n_ins
            for k in range(n_ins):
                toks_g = np.concatenate([subtiles[k * SUB + s][0] for s in range(SUB)])
                toks_e = np.concatenate([subtiles[k * SUB + s][1] for s in range(SUB)])
                toks_v = np.concatenate([subtiles[k * SUB + s][2] for s in range(SUB)])
                toks_s = np.concatenate([subtiles[k * SUB + s][3] for s in range(SUB)])
                out.append((cd, cs, toks_g, toks_e, toks_v, toks_s))
    return out, counts


def _pad_instr():
    return (np.zeros(NI, np.int16), np.zeros(NI, np.int16),
            np.tile(np.arange(128, dtype=np.float32), SUB),
            np.full(NI, TROW, np.int16))


def _wrap16(tokens):
    """[NI] token list -> [128, NI//16] int16 (token j at [j%16, j//16],
    replicated into all 8 16-partition groups)."""
    m = NI // 16
    t = np.zeros((16, m), tokens.dtype)
    idx = np.arange(NI)
    t[idx % 16, idx // 16] = tokens
    return np.tile(t, (8, 1))


def _wrap128(tokens):
    """[NI] -> [128, SUB] (token j at [j%128, j//128])."""
    t = np.zeros((128, SUB), tokens.dtype)
    idx = np.arange(NI)
    t[idx % 128, idx // 128] = tokens
    return t


def _build_program(schedule):
    """schedule: list of (cd, cs) per instruction, shared by all cores."""
    import concourse.bacc as bacc
    import concourse.mybir as mybir
    import concourse.tile as tile

    G = len(schedule)
    M16 = NI // 16
    f32, bf16, i16 = mybir.dt.float32, mybir.dt.bfloat16, mybir.dt.int16
    nc = bacc.Bacc("TRN2", target_bir_lowering=False, debug=False,
                   enable_asserts=False)
    xsT = nc.dram_tensor("xsT", [128, NPAD], f32, kind="ExternalInput")
    xdT = nc.dram_tensor("xdT", [128, NPAD], f32, kind="ExternalInput")
    W = nc.dram_tensor("W", [128, HSC], f32, kind="ExternalInput")
    Wdv = nc.dram_tensor("Wdv", [128, EDC], f32, kind="ExternalInput")
    gidx = nc.dram_tensor("gidx", [G, 128, M16], i16, kind="ExternalInput")
    edidx = nc.dram_tensor("edidx", [G, 128, M16], i16, kind="ExternalInput")
    sidx = nc.dram_tensor("sidx", [G, 128, M16], i16, kind="ExternalInput")
    gval = nc.dram_tensor("gval", [G, 128, SUB], f32, kind="ExternalInput")
    iota = nc.dram_tensor("iota", [128, 128], f32, kind="ExternalInput")
    bias = nc.dram_tensor("bias", [128, 128], f32, kind="ExternalInput")
    hs_tab = nc.dram_tensor("hs_tab", [NCHUNK * 32768, HSC], f32, kind="Internal")
    ed_tab = nc.dram_tensor("ed_tab", [NCHUNK * 32768, EDC], f32, kind="Internal")
    agg = nc.dram_tensor("agg", [NCHUNK * 32768, AGC], f32, kind="Internal")
    P = nc.dram_tensor("P", [NPAD, 128], f32, kind="ExternalOutput")

    with tile.TileContext(nc) as tc:
        with tc.tile_pool(name="const", bufs=1) as cst, \
             tc.tile_pool(name="proj", bufs=4) as prj, \
             tc.tile_pool(name="pps", bufs=2, space="PSUM") as pps, \
             tc.tile_pool(name="meta", bufs=6) as mta, \
             tc.tile_pool(name="edge", bufs=3) as edg, \
             tc.tile_pool(name="eps", bufs=4, space="PSUM") as eps:
            # ---- constants
            iot = cst.tile([128, 128], f32)
            nc.sync.dma_start(out=iot[:], in_=iota.ap())
            bia = cst.tile([128, 128], f32)
            nc.sync.dma_start(out=bia[:], in_=bias.ap())
            wt = cst.tile([128, HSC], f32)
            nc.sync.dma_start(out=wt[:], in_=W.ap())
            wdv = cst.tile([128, EDC], f32)
            nc.sync.dma_start(out=wdv[:], in_=Wdv.ap())
            zt = cst.tile([128, AGC], f32)
            nc.vector.memset(zt[:], 0.0)

            # ---- projections per 128-node tile (+ zero the agg rows the
            # node pass will later read; trash/slack rows stay garbage)
            for t in range(NPAD // 128):
                cd_t, r_t = divmod(t, 255)
                if cd_t >= NCHUNK:
                    cd_t, r_t = NCHUNK - 1, t - (NCHUNK - 1) * 255
                row0 = cd_t * 32768 + r_t * 128
                xst = prj.tile([128, 128], f32, tag="xs")
                nc.sync.dma_start(out=xst[:], in_=xsT.ap()[:, t * 128:(t + 1) * 128])
                ph = pps.tile([128, HSC], f32, tag="ph")
                nc.tensor.matmul(ph[:], xst[:], wt[:], start=True, stop=True)
                hob = prj.tile([128, HSC], f32, tag="ho")
                nc.vector.tensor_copy(hob[:], ph[:])
                nc.sync.dma_start(out=hs_tab.ap()[row0:row0 + 128], in_=hob[:])
                nc.sync.dma_start(out=agg.ap()[row0:row0 + 128], in_=zt[:])

                xdt = prj.tile([128, 128], f32, tag="xd")
                nc.sync.dma_start(out=xdt[:], in_=xdT.ap()[:, t * 128:(t + 1) * 128])
                pe = pps.tile([128, EDC], f32, tag="pe")
                nc.tensor.matmul(pe[:], xdt[:], wdv[:], start=True, stop=True)
                eob = prj.tile([128, EDC], f32, tag="eo")
                nc.vector.tensor_copy(eob[:], pe[:])
                nc.sync.dma_start(out=ed_tab.ap()[row0:row0 + 128], in_=eob[:])

            # ---- edge phase
            for g, (cd, cs) in enumerate(schedule):
                it = mta.tile([128, M16], i16, tag="gi")
                nc.sync.dma_start(out=it[:], in_=gidx.ap()[g])
                et = mta.tile([128, M16], i16, tag="ei")
                nc.sync.dma_start(out=et[:], in_=edidx.ap()[g])
                st = mta.tile([128, M16], i16, tag="si")
                nc.sync.dma_start(out=st[:], in_=sidx.ap()[g])
                gvt = mta.tile([128, SUB], f32, tag="gv")
                nc.sync.dma_start(out=gvt[:], in_=gval.ap()[g])

                ht = edg.tile([128, SUB, HSC], f32, tag="ht")
                nc.gpsimd.dma_gather(
                    out_ap=ht[:], in_ap=hs_tab.ap()[cs * 32768:(cs + 1) * 32768],
                    idxs_ap=it[:], num_idxs=NI, num_idxs_reg=NI, elem_size=HSC)
                edt = edg.tile([128, SUB, EDC], f32, tag="ed")
                nc.gpsimd.dma_gather(
                    out_ap=edt[:], in_ap=ed_tab.ap()[cd * 32768:(cd + 1) * 32768],
                    idxs_ap=et[:], num_idxs=NI, num_idxs_reg=NI, elem_size=EDC)

                z = edg.tile([128, SUB, 4], f32, tag="z")
                nc.vector.tensor_tensor(out=z[:], in0=ht[:, :, 128:132],
                                        in1=edt[:, :, 0:4],
                                        op=mybir.AluOpType.add)
                p = edg.tile([128, SUB, 4], f32, tag="p")
                nc.vector.tensor_scalar_mul(p[:], z[:], 0.2)
                nc.vector.tensor_tensor(out=p[:], in0=z[:], in1=p[:],
                                        op=mybir.AluOpType.max)
                nc.scalar.activation(out=p[:], in_=p[:],
                                     func=mybir.ActivationFunctionType.Exp)
                # msg: hs scaled per head, p into cols 128..131
                for h in range(H):
                    nc.vector.tensor_tensor(
                        out=ht[:, :, h * 32:(h + 1) * 32],
                        in0=ht[:, :, h * 32:(h + 1) * 32],
                        in1=p[:, :, h:h + 1].to_broadcast([128, SUB, 32]),
                        op=mybir.AluOpType.mult)
                nc.vector.tensor_copy(ht[:, :, 128:132], p[:])

                pay = edg.tile([128, SUB, 132], f32, tag="pay")
                for s in range(SUB):
                    S = edg.tile([128, 128], f32, tag="S")
                    nc.vector.tensor_tensor(
                        out=S[:], in0=gvt[:, s:s + 1].to_broadcast([128, 128]),
                        in1=iot[:], op=mybir.AluOpType.is_equal)
                    gp = eps.tile([128, 132], f32, tag="gp")
                    nc.tensor.matmul(gp[:], S[:], ht[:, s, 0:132], start=True, stop=True)
                    nc.vector.tensor_copy(pay[:, s, :], gp[:])
                nc.gpsimd.dma_scatter_add(
                    agg.ap()[cd * 32768:(cd + 1) * 32768, 0:132],
                    pay[:], st[:], NI, NI, 132, elem_step=AGC)

            # ---- node pass: P = agg/denom + bias
            for t in range(NPAD // 128):
                cd_t, r_t = divmod(t, 255)
                if cd_t >= NCHUNK:
                    cd_t, r_t = NCHUNK - 1, t - (NCHUNK - 1) * 255
                row0 = cd_t * 32768 + r_t * 128
                at = prj.tile([128, 132], f32, tag="at")
                nc.sync.dma_start(out=at[:], in_=agg.ap()[row0:row0 + 128, 0:132])
                dn = prj.tile([128, 4], f32, tag="dn")
                nc.vector.tensor_scalar_add(dn[:], at[:, 128:132], 1e-16)
                rc = prj.tile([128, 4], f32, tag="rc")
                nc.vector.reciprocal(rc[:], dn[:])
                ot = prj.tile([128, 128], f32, tag="ot")
                for h in range(H):
                    nc.vector.tensor_tensor(
                        out=ot[:, h * 32:(h + 1) * 32],
                        in0=at[:, h * 32:(h + 1) * 32],
                        in1=rc[:, h:h + 1].to_broadcast([128, 32]),
                        op=mybir.AluOpType.mult)
                nc.vector.tensor_tensor(out=ot[:], in0=ot[:], in1=bia[:],
                                        op=mybir.AluOpType.add)
                nc.sync.dma_start(out=P.ap()[t * 128:(t + 1) * 128], in_=ot[:])
    nc.compile()
    return nc


def _elu(x):
    return np.where(x > 0, x, np.expm1(np.minimum(x, 0.0)))


def _prep_weights(Ws, a_s, Wd, a_d):
    """-> (W [128,HSC] f32, Wdv [128,EDC] f32) for one relation."""
    W = np.zeros((128, HSC), np.float32)
    W[:, :128] = Ws
    W[:, 128:132] = np.einsum("khc,hc->kh", Ws.reshape(128, H, C), a_s)
    Wdv = np.zeros((128, EDC), np.float32)
    Wdv[:, :4] = np.einsum("khc,hc->kh", Wd.reshape(128, H, C), a_d)
    return W, Wdv


def _xT(x):
    out = np.zeros((128, NPAD), np.float32)
    out[:, :N] = np.ascontiguousarray(x.T)
    return out


def _run_layer(nc, xs, Wb, Wdvb, bvals, meta):
    """One launch over 8 cores.  Returns per-core P [N,128] f32."""
    from concourse import bass_utils
    in_maps = []
    for r, (si, di) in enumerate(REL):
        m = dict(meta[r])
        m["xsT"] = _xT(xs[si])
        m["xdT"] = _xT(xs[di])
        m["W"] = Wb[r]
        m["Wdv"] = Wdvb[r]
        m["bias"] = np.tile(bvals[r][None, :], (128, 1)).astype(np.float32)
        in_maps.append(m)
    res = bass_utils.run_bass_kernel_spmd(nc, in_maps, core_ids=list(range(8)))
    return [res.results[r]["P"][:N] for r in range(R)]


def _combine(parts):
    """Per-type sums of per-relation partials -> xs list (pre-ELU)."""
    xs = [np.zeros((N, 128), np.float32) for _ in range(5)]
    for r, (si, di) in enumerate(REL):
        xs[di] += parts[r]
    return xs


def _device_path(xs0, edges, Ws1, Wd1, as1, ad1, b1, Ws2, Wd2, as2, ad2, b2):
    if "prog" not in _CACHE:
        packs = []
        counts = []
        for r in range(R):
            pk, ct = _pack_relation(edges[r, 0].astype(np.int64),
                                    edges[r, 1].astype(np.int64))
            packs.append(pk)
            counts.append(ct)
        # uniform per-bin instruction counts across relations
        bins = [(cd, cs) for cd in range(NCHUNK) for cs in range(NCHUNK)]
        gmax = {b: max(ct[b] for ct in counts) for b in bins}
        schedule = []
        for b in bins:
            schedule += [b] * gmax[b]
        G = len(schedule)
        meta = []
        for r in range(R):
            by_bin = {b: [] for b in bins}
            for (cd, cs, tg, te, tv, ts) in packs[r]:
                by_bin[(cd, cs)].append((tg, te, tv, ts))
            gi = np.zeros((G, 128, NI // 16), np.int16)
            ei = np.zeros((G, 128, NI // 16), np.int16)
            si_ = np.zeros((G, 128, NI // 16), np.int16)
            gv = np.zeros((G, 128, SUB), np.float32)
            g = 0
            for b in bins:
                lst = by_bin[b]
                while len(lst) < gmax[b]:
                    lst.append(_pad_instr())
                for (tg, te, tv, ts) in lst:
                    gi[g] = _wrap16(tg)
                    ei[g] = _wrap16(te)
                    si_[g] = _wrap16(ts)
                    gv[g] = _wrap128(tv)
                    g += 1
            meta.append({"gidx": gi, "edidx": ei, "sidx": si_, "gval": gv,
                         "iota": np.tile(np.arange(128, dtype=np.float32)[None, :],
                                         (128, 1))})
        _CACHE["prog"] = _build_program(schedule)
        _CACHE["meta"] = meta
    nc, meta = _CACHE["prog"], _CACHE["meta"]

    W1b, Wdv1b, W2b, Wdv2b = [], [], [], []
    for r in range(R):
        w, wd = _prep_weights(Ws1[r], as1[r], Wd1[r], ad1[r])
        W1b.append(w)
        Wdv1b.append(wd)
        w, wd = _prep_weights(Ws2[r], as2[r], Wd2[r], ad2[r])
        W2b.append(w)
        Wdv2b.append(wd)

    parts1 = _run_layer(nc, xs0, W1b, Wdv1b, b1, meta)
    xs1 = [_elu(h) for h in _combine(parts1)]
    parts2 = _run_layer(nc, xs1, W2b, Wdv2b, b2, meta)
    xs2 = [_elu(h) for h in _combine(parts2)]
    return np.stack(xs2).astype(np.float32)


def _host_path(xs, edges, Ws1, Wd1, as1, ad1, b1, Ws2, Wd2, as2, ad2, b2):
    def layer(xs, Ws, Wd, a_s, a_d, b):
        outs = [np.zeros((x.shape[0], D), np.float32) for x in xs]
        for r, (si, di) in enumerate(REL):
            Nd = xs[di].shape[0]
            hs = (xs[si] @ Ws[r]).reshape(-1, H, C)
            hd = (xs[di] @ Wd[r]).reshape(-1, H, C)
            es = np.einsum("nhc,hc->nh", hs, a_s[r])
            ed = np.einsum("nhc,hc->nh", hd, a_d[r])
            src = edges[r, 0].astype(np.int64)
            dst = edges[r, 1].astype(np.int64)
            zv = es[src] + ed[dst]
            logit = np.where(zv > 0, zv, 0.2 * zv)
            m = np.full((Nd, H), -np.inf, np.float32)
            np.maximum.at(m, dst, logit)
            m = np.where(np.isfinite(m), m, 0.0)
            p = np.exp(logit - m[dst])
            denom = np.zeros((Nd, H), np.float32)
            np.add.at(denom, dst, p)
            alpha = p / (denom[dst] + 1e-16)
            msg = (hs[src] * alpha[:, :, None]).reshape(-1, D)
            aggv = np.zeros((Nd, D), np.float32)
            np.add.at(aggv, dst, msg)
            outs[di] = outs[di] + aggv + b[r]
        return outs

    xs = [_elu(h) for h in layer(xs, Ws1, Wd1, as1, ad1, b1)]
    xs = [_elu(h) for h in layer(xs, Ws2, Wd2, as2, ad2, b2)]
    return np.stack(xs).astype(np.float32)


def kernel(x_transaction, x_account, x_device, x_ip, x_email, edges,
           Ws1, Wd1, as1, ad1, b1, Ws2, Wd2, as2, ad2, b2):
    xs = [np.asarray(x, np.float32) for x in
          (x_transaction, x_account, x_device, x_ip, x_email)]
    edges = np.asarray(edges)
    args = [np.asarray(a, np.float32) for a in
            (Ws1, Wd1, as1, ad1, b1, Ws2, Wd2, as2, ad2, b2)]
    try:
        return _device_path(xs, edges, *args)
    except Exception as e:
        import sys
        print(f"[kernel] device path failed ({type(e).__name__}: {e}); "
              f"falling back to host", file=sys.stderr)
        return _host_path(xs, edges, *args)


# revision 6
# speedup vs baseline: 1.3683x; 1.3683x over previous
"""Bass/Trainium2 kernel for nn_GATModel (hetero 2-layer GAT, 8 relations).

Sharding: relation r -> NeuronCore r (8 relations, 8 cores).  The whole
layer runs on device: per-core projections (hs_ext = x_si @ [Ws|Wsv] in
bf16, ed = x_di @ Wdv), then an edge phase driven by host-prepacked
dst-sorted edge streams: dma_gather of hs_ext[src] / ed[dst] rows,
p = exp(leakyrelu(es+ed)) on ACT, per-128-edge-group segment sums via
one-hot matmul on PE (host-aligned so every destination row is written
by exactly one descriptor -- dma_scatter_add collides unsafely
otherwise), and dma_scatter_add into a chunked aggregate table.  A
node-level pass divides by the attention denominator and adds bias.
The inter-relation sum + ELU at layer boundaries runs on host between
the two launches of the same compiled program (segment-max of the
reference softmax is skipped: logits here are O(1) so the max-shift is
a mathematical no-op).

Self-contained: shapes/relations hardcoded; no sibling imports.
"""
import numpy as np

N = 100000
IN = 128
H = 4
C = 32
D = H * C
R = 8
E = 300000
REL = [(0, 1), (1, 0), (0, 2), (2, 0), (0, 3), (3, 0), (0, 4), (4, 0)]

CHUNK = 32640          # real node rows per int16-indexed table chunk (255*128)
TROW = 32767           # trash row inside each 32768-row chunk
NCHUNK = 4             # ceil(100000/32640) = 4
NI = 896               # tokens per SWDGE instruction (64-desc/lane limit)
SUB = NI // 128        # 7 subtiles per instruction
HSC = 256              # hs_ext table row elems (bf16): 128 hs + 4 es + pad
EDC = 64               # ed table row elems (fp32): 4 ed + pad
AGC = 192              # aggregate table row stride (fp32): 128 msg + 4 denom
NPAD = 782 * 128       # 100096, node count padded to 128

_CACHE = {}


def _pack_relation(src, dst):
    """Pack one relation's edges into per-instruction metadata.

    Returns list of (cd, cs, gidx[896]i16, edidx[896]i16, gval[896]f32,
    sidx[896]i16) in a fixed (cd, cs) bin order, plus per-bin instruction
    counts.  Within each bin edges are dst-sorted and whole dst-groups
    are packed into 128-token subtiles so each dst row is summed by one
    PE matmul and scattered exactly once.
    """
    order = np.argsort(dst, kind="stable")
    src, dst = src[order], dst[order]
    cd_all = dst // CHUNK
    cs_all = src // CHUNK
    out = []
    counts = {}
    for cd in range(NCHUNK):
        m_cd = cd_all == cd
        for cs in range(NCHUNK):
            m = m_cd & (cs_all == cs)
            s_loc = (src[m] - cs * CHUNK).astype(np.int16)
            d_loc = (dst[m] - cd * CHUNK).astype(np.int16)
            # dst-groups (d_loc sorted within bin)
            subtiles = []  # each: (g[128]i16 gidx, e[128]i16, gv[128], si[128])
            gi = np.zeros(128, np.int16)
            ei = np.zeros(128, np.int16)
            gv = np.arange(128, dtype=np.float32)
            si = np.full(128, TROW, np.int16)
            fill = 0
            ngr = 0
            bnd = np.nonzero(np.diff(d_loc))[0] + 1
            starts = np.concatenate(([0], bnd))
            ends = np.concatenate((bnd, [len(d_loc)]))
            for a, b in zip(starts, ends):
                gsz = b - a
                assert gsz <= 128, "dst group too large for a subtile"
                if fill + gsz > 128:
                    subtiles.append((gi, ei, gv, si))
                    gi = np.zeros(128, np.int16)
                    ei = np.zeros(128, np.int16)
                    gv = np.arange(128, dtype=np.float32)
                    si = np.full(128, TROW, np.int16)
                    fill = 0
                    ngr = 0
                gi[fill:fill + gsz] = s_loc[a:b]
                ei[fill:fill + gsz] = d_loc[a:b]
                gv[fill:fill + gsz] = ngr
                si[ngr] = d_loc[a]
                fill += gsz
                ngr += 1
            if fill or not subtiles:
                subtiles.append((gi, ei, gv, si))
            # pad subtile list to a multiple of SUB
            while len(subtiles) % SUB:
                subtiles.append((np.zeros(128, np.int16), np.zeros(128, np.int16),
                                 np.arange(128, dtype=np.float32),
                                 np.full(128, TROW, np.int16)))
            n_ins = len(subtiles) // SUB
            counts[(cd, cs)] = n_ins
            for k in range(n_ins):
                toks_g = np.concatenate([subtiles[k * SUB + s][0] for s in range(SUB)])
                toks_e = np.concatenate([subtiles[k * SUB + s][1] for s in range(SUB)])
                toks_v = np.concatenate([subtiles[k * SUB + s][2] for s in range(SUB)])
                toks_s = np.concatenate([subtiles[k * SUB + s][3] for s in range(SUB)])
                out.append((cd, cs, toks_g, toks_e, toks_v, toks_s))
    return out, counts


def _pad_instr():
    return (np.zeros(NI, np.int16), np.zeros(NI, np.int16),
            np.tile(np.arange(128, dtype=np.float32), SUB),
            np.full(NI, TROW, np.int16))


def _wrap16(tokens):
    """[NI] token list -> [128, NI//16] int16 (token j at [j%16, j//16],
    replicated into all 8 16-partition groups)."""
    m = NI // 16
    t = np.zeros((16, m), tokens.dtype)
    idx = np.arange(NI)
    t[idx % 16, idx // 16] = tokens
    return np.tile(t, (8, 1))


def _wrap128(tokens):
    """[NI] -> [128, SUB] (token j at [j%128, j//128])."""
    t = np.zeros((128, SUB), tokens.dtype)
    idx = np.arange(NI)
    t[idx % 128, idx // 128] = tokens
    return t


def _build_program(schedule):
    """schedule: list of (cd, cs) per instruction, shared by all cores."""
    import concourse.bacc as bacc
    import concourse.mybir as mybir
    import concourse.tile as tile

    G = len(schedule)
    M16 = NI // 16
    f32, bf16, i16 = mybir.dt.float32, mybir.dt.bfloat16, mybir.dt.int16
    nc = bacc.Bacc("TRN2", target_bir_lowering=False, debug=False,
                   enable_asserts=False)
    xsT = nc.dram_tensor("xsT", [128, NPAD], bf16, kind="ExternalInput")
    xdT = nc.dram_tensor("xdT", [128, NPAD], bf16, kind="ExternalInput")
    W = nc.dram_tensor("W", [128, HSC], bf16, kind="ExternalInput")
    Wdv = nc.dram_tensor("Wdv", [128, EDC], bf16, kind="ExternalInput")
    gidx = nc.dram_tensor("gidx", [G, 128, M16], i16, kind="ExternalInput")
    edidx = nc.dram_tensor("edidx", [G, 128, M16], i16, kind="ExternalInput")
    sidx = nc.dram_tensor("sidx", [G, 128, M16], i16, kind="ExternalInput")
    gval = nc.dram_tensor("gval", [G, 128, SUB], f32, kind="ExternalInput")
    iota = nc.dram_tensor("iota", [128, 128], f32, kind="ExternalInput")
    bias = nc.dram_tensor("bias", [128, 128], f32, kind="ExternalInput")
    hs_tab = nc.dram_tensor("hs_tab", [NCHUNK * 32768, HSC], bf16, kind="Internal")
    ed_tab = nc.dram_tensor("ed_tab", [NCHUNK * 32768, EDC], f32, kind="Internal")
    agg = nc.dram_tensor("agg", [NCHUNK * 32768, AGC], f32, kind="Internal")
    P = nc.dram_tensor("P", [NPAD, 128], f32, kind="ExternalOutput")

    with tile.TileContext(nc) as tc:
        with tc.tile_pool(name="const", bufs=1) as cst, \
             tc.tile_pool(name="proj", bufs=4) as prj, \
             tc.tile_pool(name="pps", bufs=2, space="PSUM") as pps, \
             tc.tile_pool(name="meta", bufs=6) as mta, \
             tc.tile_pool(name="edge", bufs=3) as edg, \
             tc.tile_pool(name="eps", bufs=4, space="PSUM") as eps:
            # ---- constants
            iot = cst.tile([128, 128], f32)
            nc.sync.dma_start(out=iot[:], in_=iota.ap())
            bia = cst.tile([128, 128], f32)
            nc.sync.dma_start(out=bia[:], in_=bias.ap())
            wt = cst.tile([128, HSC], bf16)
            nc.sync.dma_start(out=wt[:], in_=W.ap())
            wdv = cst.tile([128, EDC], bf16)
            nc.sync.dma_start(out=wdv[:], in_=Wdv.ap())
            zt = cst.tile([128, AGC], f32)
            nc.vector.memset(zt[:], 0.0)

            # ---- projections per 128-node tile (+ zero the agg rows the
            # node pass will later read; trash/slack rows stay garbage)
            for t in range(NPAD // 128):
                cd_t, r_t = divmod(t, 255)
                if cd_t >= NCHUNK:
                    cd_t, r_t = NCHUNK - 1, t - (NCHUNK - 1) * 255
                row0 = cd_t * 32768 + r_t * 128
                xst = prj.tile([128, 128], bf16, tag="xs")
                nc.sync.dma_start(out=xst[:], in_=xsT.ap()[:, t * 128:(t + 1) * 128])
                ph = pps.tile([128, HSC], f32, tag="ph")
                nc.tensor.matmul(ph[:], xst[:], wt[:], start=True, stop=True)
                hob = prj.tile([128, HSC], bf16, tag="ho")
                nc.vector.tensor_copy(hob[:], ph[:])
                nc.sync.dma_start(out=hs_tab.ap()[row0:row0 + 128], in_=hob[:])
                nc.sync.dma_start(out=agg.ap()[row0:row0 + 128], in_=zt[:])

                xdt = prj.tile([128, 128], bf16, tag="xd")
                nc.sync.dma_start(out=xdt[:], in_=xdT.ap()[:, t * 128:(t + 1) * 128])
                pe = pps.tile([128, EDC], f32, tag="pe")
                nc.tensor.matmul(pe[:], xdt[:], wdv[:], start=True, stop=True)
                eob = prj.tile([128, EDC], f32, tag="eo")
                nc.vector.tensor_copy(eob[:], pe[:])
                nc.sync.dma_start(out=ed_tab.ap()[row0:row0 + 128], in_=eob[:])

            # ---- edge phase
            for g, (cd, cs) in enumerate(schedule):
                it = mta.tile([128, M16], i16, tag="gi")
                nc.sync.dma_start(out=it[:], in_=gidx.ap()[g])
                et = mta.tile([128, M16], i16, tag="ei")
                nc.sync.dma_start(out=et[:], in_=edidx.ap()[g])
                st = mta.tile([128, M16], i16, tag="si")
                nc.sync.dma_start(out=st[:], in_=sidx.ap()[g])
                gvt = mta.tile([128, SUB], f32, tag="gv")
                nc.sync.dma_start(out=gvt[:], in_=gval.ap()[g])

                ht = edg.tile([128, SUB, HSC], bf16, tag="ht")
                nc.gpsimd.dma_gather(
                    out_ap=ht[:], in_ap=hs_tab.ap()[cs * 32768:(cs + 1) * 32768],
                    idxs_ap=it[:], num_idxs=NI, num_idxs_reg=NI, elem_size=HSC)
                edt = edg.tile([128, SUB, EDC], f32, tag="ed")
                nc.gpsimd.dma_gather(
                    out_ap=edt[:], in_ap=ed_tab.ap()[cd * 32768:(cd + 1) * 32768],
                    idxs_ap=et[:], num_idxs=NI, num_idxs_reg=NI, elem_size=EDC)

                hf = edg.tile([128, SUB, 132], f32, tag="hf")
                nc.vector.tensor_copy(hf[:], ht[:, :, 0:132])
                z = edg.tile([128, SUB, 4], f32, tag="z")
                nc.vector.tensor_tensor(out=z[:], in0=hf[:, :, 128:132],
                                        in1=edt[:, :, 0:4],
                                        op=mybir.AluOpType.add)
                p = edg.tile([128, SUB, 4], f32, tag="p")
                nc.vector.tensor_scalar_mul(p[:], z[:], 0.2)
                nc.vector.tensor_tensor(out=p[:], in0=z[:], in1=p[:],
                                        op=mybir.AluOpType.max)
                nc.scalar.activation(out=p[:], in_=p[:],
                                     func=mybir.ActivationFunctionType.Exp)
                # msg: hs scaled per head, p into cols 128..131
                for h in range(H):
                    nc.vector.tensor_tensor(
                        out=hf[:, :, h * 32:(h + 1) * 32],
                        in0=hf[:, :, h * 32:(h + 1) * 32],
                        in1=p[:, :, h:h + 1].to_broadcast([128, SUB, 32]),
                        op=mybir.AluOpType.mult)
                nc.vector.tensor_copy(hf[:, :, 128:132], p[:])

                pay = edg.tile([128, SUB, 132], f32, tag="pay")
                for s in range(SUB):
                    S = edg.tile([128, 128], f32, tag="S")
                    nc.vector.tensor_tensor(
                        out=S[:], in0=gvt[:, s:s + 1].to_broadcast([128, 128]),
                        in1=iot[:], op=mybir.AluOpType.is_equal)
                    gp = eps.tile([128, 132], f32, tag="gp")
                    nc.tensor.matmul(gp[:], S[:], hf[:, s, :], start=True, stop=True)
                    nc.vector.tensor_copy(pay[:, s, :], gp[:])
                nc.gpsimd.dma_scatter_add(
                    agg.ap()[cd * 32768:(cd + 1) * 32768, 0:132],
                    pay[:], st[:], NI, NI, 132, elem_step=AGC)

            # ---- node pass: P = agg/denom + bias
            for t in range(NPAD // 128):
                cd_t, r_t = divmod(t, 255)
                if cd_t >= NCHUNK:
                    cd_t, r_t = NCHUNK - 1, t - (NCHUNK - 1) * 255
                row0 = cd_t * 32768 + r_t * 128
                at = prj.tile([128, 132], f32, tag="at")
                nc.sync.dma_start(out=at[:], in_=agg.ap()[row0:row0 + 128, 0:132])
                dn = prj.tile([128, 4], f32, tag="dn")
                nc.vector.tensor_scalar_add(dn[:], at[:, 128:132], 1e-16)
                rc = prj.tile([128, 4], f32, tag="rc")
                nc.vector.reciprocal(rc[:], dn[:])
                ot = prj.tile([128, 128], f32, tag="ot")
                for h in range(H):
                    nc.vector.tensor_tensor(
                        out=ot[:, h * 32:(h + 1) * 32],
                        in0=at[:, h * 32:(h + 1) * 32],
                        in1=rc[:, h:h + 1].to_broadcast([128, 32]),
                        op=mybir.AluOpType.mult)
                nc.vector.tensor_tensor(out=ot[:], in0=ot[:], in1=bia[:],
                                        op=mybir.AluOpType.add)
                nc.sync.dma_start(out=P.ap()[t * 128:(t + 1) * 128], in_=ot[:])
    nc.compile()
    return nc


def _elu(x):
    return np.where(x > 0, x, np.expm1(np.minimum(x, 0.0)))


def _prep_weights(Ws, a_s, Wd, a_d):
    """-> (W [128,HSC] f32, Wdv [128,EDC] f32) for one relation."""
    import ml_dtypes
    W = np.zeros((128, HSC), np.float32)
    W[:, :128] = Ws
    W[:, 128:132] = np.einsum("khc,hc->kh", Ws.reshape(128, H, C), a_s)
    Wdv = np.zeros((128, EDC), np.float32)
    Wdv[:, :4] = np.einsum("khc,hc->kh", Wd.reshape(128, H, C), a_d)
    return W.astype(ml_dtypes.bfloat16), Wdv.astype(ml_dtypes.bfloat16)


def _xT(x):
    import ml_dtypes
    out = np.zeros((128, NPAD), ml_dtypes.bfloat16)
    out[:, :N] = np.ascontiguousarray(x.T).astype(ml_dtypes.bfloat16)
    return out


def _run_layer(nc, xs, Wb, Wdvb, bvals, meta):
    """One launch over 8 cores.  Returns per-core P [N,128] f32."""
    from concourse import bass_utils
    in_maps = []
    for r, (si, di) in enumerate(REL):
        m = dict(meta[r])
        m["xsT"] = _xT(xs[si])
        m["xdT"] = _xT(xs[di])
        m["W"] = Wb[r]
        m["Wdv"] = Wdvb[r]
        m["bias"] = np.tile(bvals[r][None, :], (128, 1)).astype(np.float32)
        in_maps.append(m)
    res = bass_utils.run_bass_kernel_spmd(nc, in_maps, core_ids=list(range(8)))
    return [res.results[r]["P"][:N] for r in range(R)]


def _combine(parts):
    """Per-type sums of per-relation partials -> xs list (pre-ELU)."""
    xs = [np.zeros((N, 128), np.float32) for _ in range(5)]
    for r, (si, di) in enumerate(REL):
        xs[di] += parts[r]
    return xs


def _device_path(xs0, edges, Ws1, Wd1, as1, ad1, b1, Ws2, Wd2, as2, ad2, b2):
    if "prog" not in _CACHE:
        packs = []
        counts = []
        for r in range(R):
            pk, ct = _pack_relation(edges[r, 0].astype(np.int64),
                                    edges[r, 1].astype(np.int64))
            packs.append(pk)
            counts.append(ct)
        # uniform per-bin instruction counts across relations
        bins = [(cd, cs) for cd in range(NCHUNK) for cs in range(NCHUNK)]
        gmax = {b: max(ct[b] for ct in counts) for b in bins}
        schedule = []
        for b in bins:
            schedule += [b] * gmax[b]
        G = len(schedule)
        meta = []
        for r in range(R):
            by_bin = {b: [] for b in bins}
            for (cd, cs, tg, te, tv, ts) in packs[r]:
                by_bin[(cd, cs)].append((tg, te, tv, ts))
            gi = np.zeros((G, 128, NI // 16), np.int16)
            ei = np.zeros((G, 128, NI // 16), np.int16)
            si_ = np.zeros((G, 128, NI // 16), np.int16)
            gv = np.zeros((G, 128, SUB), np.float32)
            g = 0
            for b in bins:
                lst = by_bin[b]
                while len(lst) < gmax[b]:
                    lst.append(_pad_instr())
                for (tg, te, tv, ts) in lst:
                    gi[g] = _wrap16(tg)
                    ei[g] = _wrap16(te)
                    si_[g] = _wrap16(ts)
                    gv[g] = _wrap128(tv)
                    g += 1
            meta.append({"gidx": gi, "edidx": ei, "sidx": si_, "gval": gv,
                         "iota": np.tile(np.arange(128, dtype=np.float32)[None, :],
                                         (128, 1))})
        _CACHE["prog"] = _build_program(schedule)
        _CACHE["meta"] = meta
    nc, meta = _CACHE["prog"], _CACHE["meta"]

    W1b, Wdv1b, W2b, Wdv2b = [], [], [], []
    for r in range(R):
        w, wd = _prep_weights(Ws1[r], as1[r], Wd1[r], ad1[r])
        W1b.append(w)
        Wdv1b.append(wd)
        w, wd = _prep_weights(Ws2[r], as2[r], Wd2[r], ad2[r])
        W2b.append(w)
        Wdv2b.append(wd)

    parts1 = _run_layer(nc, xs0, W1b, Wdv1b, b1, meta)
    xs1 = [_elu(h) for h in _combine(parts1)]
    parts2 = _run_layer(nc, xs1, W2b, Wdv2b, b2, meta)
    xs2 = [_elu(h) for h in _combine(parts2)]
    return np.stack(xs2).astype(np.float32)


def _host_path(xs, edges, Ws1, Wd1, as1, ad1, b1, Ws2, Wd2, as2, ad2, b2):
    def layer(xs, Ws, Wd, a_s, a_d, b):
        outs = [np.zeros((x.shape[0], D), np.float32) for x in xs]
        for r, (si, di) in enumerate(REL):
            Nd = xs[di].shape[0]
            hs = (xs[si] @ Ws[r]).reshape(-1, H, C)
            hd = (xs[di] @ Wd[r]).reshape(-1, H, C)
            es = np.einsum("nhc,hc->nh", hs, a_s[r])
            ed = np.einsum("nhc,hc->nh", hd, a_d[r])
            src = edges[r, 0].astype(np.int64)
            dst = edges[r, 1].astype(np.int64)
            zv = es[src] + ed[dst]
            logit = np.where(zv > 0, zv, 0.2 * zv)
            m = np.full((Nd, H), -np.inf, np.float32)
            np.maximum.at(m, dst, logit)
            m = np.where(np.isfinite(m), m, 0.0)
            p = np.exp(logit - m[dst])
            denom = np.zeros((Nd, H), np.float32)
            np.add.at(denom, dst, p)
            alpha = p / (denom[dst] + 1e-16)
            msg = (hs[src] * alpha[:, :, None]).reshape(-1, D)
            aggv = np.zeros((Nd, D), np.float32)
            np.add.at(aggv, dst, msg)
            outs[di] = outs[di] + aggv + b[r]
        return outs

    xs = [_elu(h) for h in layer(xs, Ws1, Wd1, as1, ad1, b1)]
    xs = [_elu(h) for h in layer(xs, Ws2, Wd2, as2, ad2, b2)]
    return np.stack(xs).astype(np.float32)


def kernel(x_transaction, x_account, x_device, x_ip, x_email, edges,
           Ws1, Wd1, as1, ad1, b1, Ws2, Wd2, as2, ad2, b2):
    xs = [np.asarray(x, np.float32) for x in
          (x_transaction, x_account, x_device, x_ip, x_email)]
    edges = np.asarray(edges)
    args = [np.asarray(a, np.float32) for a in
            (Ws1, Wd1, as1, ad1, b1, Ws2, Wd2, as2, ad2, b2)]
    try:
        return _device_path(xs, edges, *args)
    except Exception as e:
        import sys
        print(f"[kernel] device path failed ({type(e).__name__}: {e}); "
              f"falling back to host", file=sys.stderr)
        return _host_path(xs, edges, *args)


# revision 7
# speedup vs baseline: 1.6501x; 1.2060x over previous
"""Bass/Trainium2 kernel for nn_GATModel (hetero 2-layer GAT, 8 relations).

Sharding: relation r -> NeuronCore r (8 relations, 8 cores).  The whole
layer runs on device: per-core projections (hs_ext = x_si @ [Ws|Wsv] in
bf16, ed = x_di @ Wdv), then an edge phase driven by host-prepacked
dst-sorted edge streams: dma_gather of hs_ext[src] / ed[dst] rows,
p = exp(leakyrelu(es+ed)) on ACT, per-128-edge-group segment sums via
one-hot matmul on PE (host-aligned so every destination row is written
by exactly one descriptor -- dma_scatter_add collides unsafely
otherwise), and dma_scatter_add into a chunked aggregate table.  A
node-level pass divides by the attention denominator and adds bias.
The inter-relation sum + ELU at layer boundaries runs on host between
the two launches of the same compiled program (segment-max of the
reference softmax is skipped: logits here are O(1) so the max-shift is
a mathematical no-op).

Self-contained: shapes/relations hardcoded; no sibling imports.
"""
import numpy as np

N = 100000
IN = 128
H = 4
C = 32
D = H * C
R = 8
E = 300000
REL = [(0, 1), (1, 0), (0, 2), (2, 0), (0, 3), (3, 0), (0, 4), (4, 0)]

CHUNK = 32640          # real node rows per int16-indexed table chunk (255*128)
TROW = 32767           # trash row inside each 32768-row chunk
NCHUNK = 4             # ceil(100000/32640) = 4
NI = 896               # tokens per SWDGE instruction (64-desc/lane limit)
SUB = NI // 128        # 7 subtiles per instruction
HSC = 256              # hs_ext table row elems (bf16): 128 hs + 4 es + pad
EDC = 64               # ed table row elems (fp32): 4 ed + pad
AGC = 192              # aggregate table row stride (fp32): 128 msg + 4 denom
NPAD = 782 * 128       # 100096, node count padded to 128

_CACHE = {}


def _pack_relation(src, dst):
    """Pack one relation's edges into per-instruction metadata.

    Returns list of (cd, cs, gidx[896]i16, edidx[896]i16, gval[896]f32,
    sidx[896]i16) in a fixed (cd, cs) bin order, plus per-bin instruction
    counts.  Within each bin edges are dst-sorted and whole dst-groups
    are packed into 128-token subtiles so each dst row is summed by one
    PE matmul and scattered exactly once.
    """
    order = np.argsort(dst, kind="stable")
    src, dst = src[order], dst[order]
    cd_all = dst // CHUNK
    cs_all = src // CHUNK
    out = []
    counts = {}
    for cd in range(NCHUNK):
        m_cd = cd_all == cd
        for cs in range(NCHUNK):
            m = m_cd & (cs_all == cs)
            s_loc = (src[m] - cs * CHUNK).astype(np.int16)
            d_loc = (dst[m] - cd * CHUNK).astype(np.int16)
            # dst-groups (d_loc sorted within bin)
            subtiles = []  # each: (g[128]i16 gidx, e[128]i16, gv[128], si[128])
            gi = np.zeros(128, np.int16)
            ei = np.zeros(128, np.int16)
            gv = np.arange(128, dtype=np.float32)
            si = np.full(128, TROW, np.int16)
            fill = 0
            ngr = 0
            bnd = np.nonzero(np.diff(d_loc))[0] + 1
            starts = np.concatenate(([0], bnd))
            ends = np.concatenate((bnd, [len(d_loc)]))
            for a, b in zip(starts, ends):
                gsz = b - a
                assert gsz <= 128, "dst group too large for a subtile"
                if fill + gsz > 128:
                    subtiles.append((gi, ei, gv, si))
                    gi = np.zeros(128, np.int16)
                    ei = np.zeros(128, np.int16)
                    gv = np.arange(128, dtype=np.float32)
                    si = np.full(128, TROW, np.int16)
                    fill = 0
                    ngr = 0
                gi[fill:fill + gsz] = s_loc[a:b]
                ei[fill:fill + gsz] = d_loc[a:b]
                gv[fill:fill + gsz] = ngr
                si[ngr] = d_loc[a]
                fill += gsz
                ngr += 1
            if fill or not subtiles:
                subtiles.append((gi, ei, gv, si))
            # pad subtile list to a multiple of SUB
            while len(subtiles) % SUB:
                subtiles.append((np.zeros(128, np.int16), np.zeros(128, np.int16),
                                 np.arange(128, dtype=np.float32),
                                 np.full(128, TROW, np.int16)))
            n_ins = len(subtiles) // SUB
            counts[(cd, cs)] = n_ins
            for k in range(n_ins):
                toks_g = np.concatenate([subtiles[k * SUB + s][0] for s in range(SUB)])
                toks_e = np.concatenate([subtiles[k * SUB + s][1] for s in range(SUB)])
                toks_v = np.concatenate([subtiles[k * SUB + s][2] for s in range(SUB)])
                toks_s = np.concatenate([subtiles[k * SUB + s][3] for s in range(SUB)])
                out.append((cd, cs, toks_g, toks_e, toks_v, toks_s))
    return out, counts


def _pad_instr():
    return (np.zeros(NI, np.int16), np.zeros(NI, np.int16),
            np.tile(np.arange(128, dtype=np.float32), SUB),
            np.full(NI, TROW, np.int16))


def _wrap16(tokens):
    """[NI] token list -> [128, NI//16] int16 (token j at [j%16, j//16],
    replicated into all 8 16-partition groups)."""
    m = NI // 16
    t = np.zeros((16, m), tokens.dtype)
    idx = np.arange(NI)
    t[idx % 16, idx // 16] = tokens
    return np.tile(t, (8, 1))


def _wrap128(tokens):
    """[NI] -> [128, SUB] (token j at [j%128, j//128])."""
    t = np.zeros((128, SUB), tokens.dtype)
    idx = np.arange(NI)
    t[idx % 128, idx // 128] = tokens
    return t


def _build_program(schedule):
    """schedule: list of (cd, cs) per instruction, shared by all cores."""
    import concourse.bacc as bacc
    import concourse.mybir as mybir
    import concourse.tile as tile

    G = len(schedule)
    M16 = NI // 16
    f32, bf16, i16 = mybir.dt.float32, mybir.dt.bfloat16, mybir.dt.int16
    nc = bacc.Bacc("TRN2", target_bir_lowering=False, debug=False,
                   enable_asserts=False)
    xsT = nc.dram_tensor("xsT", [128, NPAD], bf16, kind="ExternalInput")
    xdT = nc.dram_tensor("xdT", [128, NPAD], bf16, kind="ExternalInput")
    W = nc.dram_tensor("W", [128, HSC], bf16, kind="ExternalInput")
    Wdv = nc.dram_tensor("Wdv", [128, EDC], bf16, kind="ExternalInput")
    idx3 = nc.dram_tensor("idx3", [G, 128, 3 * M16], i16, kind="ExternalInput")
    gval = nc.dram_tensor("gval", [G, 128, SUB], f32, kind="ExternalInput")
    iota = nc.dram_tensor("iota", [128, 128], f32, kind="ExternalInput")
    bias = nc.dram_tensor("bias", [128, 128], f32, kind="ExternalInput")
    hs_tab = nc.dram_tensor("hs_tab", [NCHUNK * 32768, HSC], bf16, kind="Internal")
    ed_tab = nc.dram_tensor("ed_tab", [NCHUNK * 32768, EDC], f32, kind="Internal")
    agg = nc.dram_tensor("agg", [NCHUNK * 32768, AGC], f32, kind="Internal")
    P = nc.dram_tensor("P", [NPAD, 128], bf16, kind="ExternalOutput")

    with tile.TileContext(nc) as tc:
        with tc.tile_pool(name="const", bufs=1) as cst, \
             tc.tile_pool(name="proj", bufs=4) as prj, \
             tc.tile_pool(name="pps", bufs=2, space="PSUM") as pps, \
             tc.tile_pool(name="meta", bufs=6) as mta, \
             tc.tile_pool(name="edge", bufs=3) as edg, \
             tc.tile_pool(name="eps", bufs=4, space="PSUM") as eps:
            # ---- constants
            iot = cst.tile([128, 128], f32)
            nc.sync.dma_start(out=iot[:], in_=iota.ap())
            bia = cst.tile([128, 128], f32)
            nc.sync.dma_start(out=bia[:], in_=bias.ap())
            wt = cst.tile([128, HSC], bf16)
            nc.sync.dma_start(out=wt[:], in_=W.ap())
            wdv = cst.tile([128, EDC], bf16)
            nc.sync.dma_start(out=wdv[:], in_=Wdv.ap())
            zt = cst.tile([128, AGC], f32)
            nc.vector.memset(zt[:], 0.0)

            # ---- projections per 128-node tile (+ zero the agg rows the
            # node pass will later read; trash/slack rows stay garbage)
            for t in range(NPAD // 128):
                cd_t, r_t = divmod(t, 255)
                if cd_t >= NCHUNK:
                    cd_t, r_t = NCHUNK - 1, t - (NCHUNK - 1) * 255
                row0 = cd_t * 32768 + r_t * 128
                xst = prj.tile([128, 128], bf16, tag="xs")
                nc.sync.dma_start(out=xst[:], in_=xsT.ap()[:, t * 128:(t + 1) * 128])
                ph = pps.tile([128, HSC], f32, tag="ph")
                nc.tensor.matmul(ph[:], xst[:], wt[:], start=True, stop=True)
                hob = prj.tile([128, HSC], bf16, tag="ho")
                nc.vector.tensor_copy(hob[:], ph[:])
                nc.sync.dma_start(out=hs_tab.ap()[row0:row0 + 128], in_=hob[:])
                nc.sync.dma_start(out=agg.ap()[row0:row0 + 128], in_=zt[:])

                xdt = prj.tile([128, 128], bf16, tag="xd")
                nc.sync.dma_start(out=xdt[:], in_=xdT.ap()[:, t * 128:(t + 1) * 128])
                pe = pps.tile([128, EDC], f32, tag="pe")
                nc.tensor.matmul(pe[:], xdt[:], wdv[:], start=True, stop=True)
                eob = prj.tile([128, EDC], f32, tag="eo")
                nc.vector.tensor_copy(eob[:], pe[:])
                nc.sync.dma_start(out=ed_tab.ap()[row0:row0 + 128], in_=eob[:])

            # ---- edge phase
            for g, (cd, cs) in enumerate(schedule):
                i3 = mta.tile([128, 3 * M16], i16, tag="i3")
                nc.sync.dma_start(out=i3[:], in_=idx3.ap()[g])
                it = i3[:, 0:M16]
                et = i3[:, M16:2 * M16]
                st = i3[:, 2 * M16:3 * M16]
                gvt = mta.tile([128, SUB], f32, tag="gv")
                nc.sync.dma_start(out=gvt[:], in_=gval.ap()[g])

                ht = edg.tile([128, SUB, HSC], bf16, tag="ht")
                nc.gpsimd.dma_gather(
                    out_ap=ht[:], in_ap=hs_tab.ap()[cs * 32768:(cs + 1) * 32768],
                    idxs_ap=it, num_idxs=NI, num_idxs_reg=NI, elem_size=HSC)
                edt = edg.tile([128, SUB, EDC], f32, tag="ed")
                nc.gpsimd.dma_gather(
                    out_ap=edt[:], in_ap=ed_tab.ap()[cd * 32768:(cd + 1) * 32768],
                    idxs_ap=et, num_idxs=NI, num_idxs_reg=NI, elem_size=EDC)

                hf = edg.tile([128, SUB, 132], f32, tag="hf")
                nc.vector.tensor_copy(hf[:], ht[:, :, 0:132])
                z = edg.tile([128, SUB, 4], f32, tag="z")
                nc.vector.tensor_tensor(out=z[:], in0=hf[:, :, 128:132],
                                        in1=edt[:, :, 0:4],
                                        op=mybir.AluOpType.add)
                p = edg.tile([128, SUB, 4], f32, tag="p")
                nc.vector.tensor_scalar_mul(p[:], z[:], 0.2)
                nc.vector.tensor_tensor(out=p[:], in0=z[:], in1=p[:],
                                        op=mybir.AluOpType.max)
                nc.scalar.activation(out=p[:], in_=p[:],
                                     func=mybir.ActivationFunctionType.Exp)
                # msg: hs scaled per head, p into cols 128..131
                for h in range(H):
                    nc.vector.tensor_tensor(
                        out=hf[:, :, h * 32:(h + 1) * 32],
                        in0=hf[:, :, h * 32:(h + 1) * 32],
                        in1=p[:, :, h:h + 1].to_broadcast([128, SUB, 32]),
                        op=mybir.AluOpType.mult)
                nc.vector.tensor_copy(hf[:, :, 128:132], p[:])

                pay = edg.tile([128, SUB, 132], f32, tag="pay")
                for s in range(SUB):
                    S = edg.tile([128, 128], f32, tag="S")
                    nc.vector.tensor_tensor(
                        out=S[:], in0=gvt[:, s:s + 1].to_broadcast([128, 128]),
                        in1=iot[:], op=mybir.AluOpType.is_equal)
                    gp = eps.tile([128, 132], f32, tag="gp")
                    nc.tensor.matmul(gp[:], S[:], hf[:, s, :], start=True, stop=True)
                    nc.vector.tensor_copy(pay[:, s, :], gp[:])
                nc.gpsimd.dma_scatter_add(
                    agg.ap()[cd * 32768:(cd + 1) * 32768, 0:132],
                    pay[:], st, NI, NI, 132, elem_step=AGC)

            # ---- node pass: P = agg/denom + bias
            for t in range(NPAD // 128):
                cd_t, r_t = divmod(t, 255)
                if cd_t >= NCHUNK:
                    cd_t, r_t = NCHUNK - 1, t - (NCHUNK - 1) * 255
                row0 = cd_t * 32768 + r_t * 128
                at = prj.tile([128, 132], f32, tag="at")
                nc.sync.dma_start(out=at[:], in_=agg.ap()[row0:row0 + 128, 0:132])
                dn = prj.tile([128, 4], f32, tag="dn")
                nc.vector.tensor_scalar_add(dn[:], at[:, 128:132], 1e-16)
                rc = prj.tile([128, 4], f32, tag="rc")
                nc.vector.reciprocal(rc[:], dn[:])
                ot = prj.tile([128, 128], f32, tag="ot")
                for h in range(H):
                    nc.vector.tensor_tensor(
                        out=ot[:, h * 32:(h + 1) * 32],
                        in0=at[:, h * 32:(h + 1) * 32],
                        in1=rc[:, h:h + 1].to_broadcast([128, 32]),
                        op=mybir.AluOpType.mult)
                ob = prj.tile([128, 128], bf16, tag="ob")
                nc.vector.tensor_tensor(out=ob[:], in0=ot[:], in1=bia[:],
                                        op=mybir.AluOpType.add)
                nc.sync.dma_start(out=P.ap()[t * 128:(t + 1) * 128], in_=ob[:])
    nc.compile()
    return nc


def _elu(x):
    return np.where(x > 0, x, np.expm1(np.minimum(x, 0.0)))


def _prep_weights(Ws, a_s, Wd, a_d):
    """-> (W [128,HSC] f32, Wdv [128,EDC] f32) for one relation."""
    import ml_dtypes
    W = np.zeros((128, HSC), np.float32)
    W[:, :128] = Ws
    W[:, 128:132] = np.einsum("khc,hc->kh", Ws.reshape(128, H, C), a_s)
    Wdv = np.zeros((128, EDC), np.float32)
    Wdv[:, :4] = np.einsum("khc,hc->kh", Wd.reshape(128, H, C), a_d)
    return W.astype(ml_dtypes.bfloat16), Wdv.astype(ml_dtypes.bfloat16)


def _xT(x):
    import ml_dtypes
    out = np.zeros((128, NPAD), ml_dtypes.bfloat16)
    out[:, :N] = np.ascontiguousarray(x.T).astype(ml_dtypes.bfloat16)
    return out


def _run_layer(nc, xs, Wb, Wdvb, bvals, meta):
    """One launch over 8 cores.  Returns per-core P [N,128] f32."""
    from concourse import bass_utils
    in_maps = []
    for r, (si, di) in enumerate(REL):
        m = dict(meta[r])
        m["xsT"] = _xT(xs[si])
        m["xdT"] = _xT(xs[di])
        m["W"] = Wb[r]
        m["Wdv"] = Wdvb[r]
        m["bias"] = np.tile(bvals[r][None, :], (128, 1)).astype(np.float32)
        in_maps.append(m)
    res = bass_utils.run_bass_kernel_spmd(nc, in_maps, core_ids=list(range(8)))
    return [np.asarray(res.results[r]["P"][:N], np.float32) for r in range(R)]


def _combine(parts):
    """Per-type sums of per-relation partials -> xs list (pre-ELU)."""
    xs = [np.zeros((N, 128), np.float32) for _ in range(5)]
    for r, (si, di) in enumerate(REL):
        xs[di] += parts[r]
    return xs


def _device_path(xs0, edges, Ws1, Wd1, as1, ad1, b1, Ws2, Wd2, as2, ad2, b2):
    if "prog" not in _CACHE:
        packs = []
        counts = []
        for r in range(R):
            pk, ct = _pack_relation(edges[r, 0].astype(np.int64),
                                    edges[r, 1].astype(np.int64))
            packs.append(pk)
            counts.append(ct)
        # uniform per-bin instruction counts across relations
        bins = [(cd, cs) for cd in range(NCHUNK) for cs in range(NCHUNK)]
        gmax = {b: max(ct[b] for ct in counts) for b in bins}
        schedule = []
        for b in bins:
            schedule += [b] * gmax[b]
        G = len(schedule)
        meta = []
        for r in range(R):
            by_bin = {b: [] for b in bins}
            for (cd, cs, tg, te, tv, ts) in packs[r]:
                by_bin[(cd, cs)].append((tg, te, tv, ts))
            i3 = np.zeros((G, 128, 3 * (NI // 16)), np.int16)
            gv = np.zeros((G, 128, SUB), np.float32)
            M16 = NI // 16
            g = 0
            for b in bins:
                lst = by_bin[b]
                while len(lst) < gmax[b]:
                    lst.append(_pad_instr())
                for (tg, te, tv, ts) in lst:
                    i3[g, :, 0:M16] = _wrap16(tg)
                    i3[g, :, M16:2 * M16] = _wrap16(te)
                    i3[g, :, 2 * M16:3 * M16] = _wrap16(ts)
                    gv[g] = _wrap128(tv)
                    g += 1
            meta.append({"idx3": i3, "gval": gv,
                         "iota": np.tile(np.arange(128, dtype=np.float32)[None, :],
                                         (128, 1))})
        _CACHE["prog"] = _build_program(schedule)
        _CACHE["meta"] = meta
    nc, meta = _CACHE["prog"], _CACHE["meta"]

    W1b, Wdv1b, W2b, Wdv2b = [], [], [], []
    for r in range(R):
        w, wd = _prep_weights(Ws1[r], as1[r], Wd1[r], ad1[r])
        W1b.append(w)
        Wdv1b.append(wd)
        w, wd = _prep_weights(Ws2[r], as2[r], Wd2[r], ad2[r])
        W2b.append(w)
        Wdv2b.append(wd)

    parts1 = _run_layer(nc, xs0, W1b, Wdv1b, b1, meta)
    xs1 = [_elu(h) for h in _combine(parts1)]
    parts2 = _run_layer(nc, xs1, W2b, Wdv2b, b2, meta)
    xs2 = [_elu(h) for h in _combine(parts2)]
    return np.stack(xs2).astype(np.float32)


def _host_path(xs, edges, Ws1, Wd1, as1, ad1, b1, Ws2, Wd2, as2, ad2, b2):
    def layer(xs, Ws, Wd, a_s, a_d, b):
        outs = [np.zeros((x.shape[0], D), np.float32) for x in xs]
        for r, (si, di) in enumerate(REL):
            Nd = xs[di].shape[0]
            hs = (xs[si] @ Ws[r]).reshape(-1, H, C)
            hd = (xs[di] @ Wd[r]).reshape(-1, H, C)
            es = np.einsum("nhc,hc->nh", hs, a_s[r])
            ed = np.einsum("nhc,hc->nh", hd, a_d[r])
            src = edges[r, 0].astype(np.int64)
            dst = edges[r, 1].astype(np.int64)
            zv = es[src] + ed[dst]
            logit = np.where(zv > 0, zv, 0.2 * zv)
            m = np.full((Nd, H), -np.inf, np.float32)
            np.maximum.at(m, dst, logit)
            m = np.where(np.isfinite(m), m, 0.0)
            p = np.exp(logit - m[dst])
            denom = np.zeros((Nd, H), np.float32)
            np.add.at(denom, dst, p)
            alpha = p / (denom[dst] + 1e-16)
            msg = (hs[src] * alpha[:, :, None]).reshape(-1, D)
            aggv = np.zeros((Nd, D), np.float32)
            np.add.at(aggv, dst, msg)
            outs[di] = outs[di] + aggv + b[r]
        return outs

    xs = [_elu(h) for h in layer(xs, Ws1, Wd1, as1, ad1, b1)]
    xs = [_elu(h) for h in layer(xs, Ws2, Wd2, as2, ad2, b2)]
    return np.stack(xs).astype(np.float32)


def kernel(x_transaction, x_account, x_device, x_ip, x_email, edges,
           Ws1, Wd1, as1, ad1, b1, Ws2, Wd2, as2, ad2, b2):
    xs = [np.asarray(x, np.float32) for x in
          (x_transaction, x_account, x_device, x_ip, x_email)]
    edges = np.asarray(edges)
    args = [np.asarray(a, np.float32) for a in
            (Ws1, Wd1, as1, ad1, b1, Ws2, Wd2, as2, ad2, b2)]
    try:
        return _device_path(xs, edges, *args)
    except Exception as e:
        import sys
        print(f"[kernel] device path failed ({type(e).__name__}: {e}); "
              f"falling back to host", file=sys.stderr)
        return _host_path(xs, edges, *args)
